# revision 11
# baseline (speedup 1.0000x reference)
"""ChannelWiseProjection Trainium2 kernel.

out[b,c,h,w] = sum_d x[b,h,w,d] * W[c,d] + bias[c]

Strategy: data-parallel over M = b*h*w (65536 rows), 8192 rows per core.
Host pre-transposes each core's x slab to [D=512, M=8192] (K-major) so the
device sees the contraction dim on SBUF partitions with no on-chip
transpose.  Per core: out_slab[C=128, M=8192] = W^T-blocked stationary
matmuls (fp32r, 4 K-blocks accumulated in PSUM) + bias fused into the
PSUM->SBUF copy.  Output slabs are channel-major so they DMA straight out
and reassemble into [b, c, h, w] on host.
"""

import numpy as np

from concourse import bass, bacc, mybir, tile
from concourse.bass_utils import run_bass_kernel_spmd

N_CORES = 8
B, H, Wdim, D = 4, 128, 128, 512
C = 128
M_TOT = B * H * Wdim          # 65536
M_CORE = M_TOT // N_CORES     # 8192
KB = D // 128                 # 4 contraction blocks
M_CHUNK = 2048                # DMA chunk along M (4 MiB per chunk load)
M_SUB = 512                   # matmul moving size (one PSUM bank, fp32)

_NC = None


def _build():
    global _NC
    if _NC is not None:
        return _NC
    # Bacc (not raw Bass): its finalize() runs the pass pipeline that
    # splits multi-waits into EventSemaphores (TRN2 allows only one sync
    # wait per instruction) — Tile output does not compile without it.
    nc = bacc.Bacc(None)
    xt = nc.declare_dram_parameter(
        "xt", [KB, 128, M_CORE], mybir.dt.float32r, isOutput=False
    )
    wt = nc.declare_dram_parameter(
        "wt", [128, KB, C], mybir.dt.float32r, isOutput=False
    )
    bias = nc.declare_dram_parameter("bias", [C, 1], mybir.dt.float32, isOutput=False)
    out = nc.declare_dram_parameter("out", [C, M_CORE], mybir.dt.float32, isOutput=True)

    with tile.TileContext(nc) as tc:
        with (
            tc.tile_pool(name="const", bufs=1) as cpool,
            tc.tile_pool(name="x", bufs=4) as xpool,
            tc.tile_pool(name="o", bufs=4) as opool,
            tc.tile_pool(name="ps", bufs=4, space="PSUM") as pspool,
        ):
            # w/bias go over SWDGE (gpsimd) so the 8 HWDGE lanes map 1:1
            # onto the 8 big DMAs below.
            w_sb = cpool.tile([128, KB, C], mybir.dt.float32r)
            nc.gpsimd.dma_start(w_sb[:], wt[:])
            b_sb = cpool.tile([C, 1], mybir.dt.float32)
            nc.gpsimd.dma_start(b_sb[:], bias[:])

            xt_r = xt[:].rearrange("kb p m -> p kb m")
            for mc in range(M_CORE // M_CHUNK):
                x_sb = xpool.tile([128, KB, M_CHUNK], mybir.dt.float32r)
                nc.sync.dma_start(
                    x_sb[:], xt_r[:, :, mc * M_CHUNK : (mc + 1) * M_CHUNK]
                )
                o_sb = opool.tile([C, M_CHUNK], mybir.dt.float32)
                for ms in range(M_CHUNK // M_SUB):
                    ps = pspool.tile([C, M_SUB], mybir.dt.float32)
                    for kb in range(KB):
                        nc.tensor.matmul(
                            ps[:],
                            w_sb[:, kb, :],
                            x_sb[:, kb, bass.ts(ms, M_SUB)],
                            start=(kb == 0),
                            stop=(kb == KB - 1),
                        )
                    nc.vector.tensor_scalar_add(
                        o_sb[:, bass.ts(ms, M_SUB)], ps[:], b_sb[:]
                    )
                nc.sync.dma_start(
                    out[:, mc * M_CHUNK : (mc + 1) * M_CHUNK], o_sb[:]
                )
    nc.finalize()  # Bacc.finalize runs the wait-splitting compile pipeline
    _NC = nc
    return nc


LAST_RESULT = None


def kernel(x, W, b):
    global LAST_RESULT
    nc = _build()

    x = np.ascontiguousarray(np.asarray(x), dtype=np.float32)
    W = np.asarray(W, dtype=np.float32)
    b = np.asarray(b, dtype=np.float32)

    # Per-core K-major slabs: [8, D, M_CORE] -> [8, KB, 128, M_CORE]
    xt = np.ascontiguousarray(
        x.reshape(N_CORES, M_CORE, D).transpose(0, 2, 1)
    ).reshape(N_CORES, KB, 128, M_CORE)
    # Stationary weights, blocked: wt[kp, kb, c] = W[c, kb*128 + kp]
    wt = np.ascontiguousarray(W.T.reshape(KB, 128, C).transpose(1, 0, 2))
    b2 = np.ascontiguousarray(b.reshape(C, 1))

    in_maps = [{"xt": xt[i], "wt": wt, "bias": b2} for i in range(N_CORES)]
    res = run_bass_kernel_spmd(nc, in_maps, list(range(N_CORES)))
    LAST_RESULT = res

    out = np.empty((B, C, H, Wdim), dtype=np.float32)
    for i in range(N_CORES):
        slab = res.results[i]["out"]  # [C, M_CORE] over m = (h, w) for batch i//2
        bi, half = divmod(i, 2)
        out[bi, :, half * 64 : (half + 1) * 64, :] = slab.reshape(C, 64, Wdim)
    return out


# revision 14
# speedup vs baseline: 1.1664x; 1.1664x over previous
"""ChannelWiseProjection Trainium2 kernel.

out[b,c,h,w] = sum_d x[b,h,w,d] * W[c,d] + bias[c]

Strategy: data-parallel over M = b*h*w (65536 rows), 8192 rows per core.
Host pre-transposes each core's x slab to [D=512, M=8192] (K-major) so the
device sees the contraction dim on SBUF partitions with no on-chip
transpose.  Per core: out_slab[C=128, M=8192] = W^T-blocked stationary
matmuls (fp32r, 4 K-blocks accumulated in PSUM) + bias fused into the
PSUM->SBUF copy.  Output slabs are channel-major so they DMA straight out
and reassemble into [b, c, h, w] on host.
"""

import numpy as np

from concourse import bass, bacc, mybir, tile
from concourse.bass_utils import run_bass_kernel_spmd

N_CORES = 8
B, H, Wdim, D = 4, 128, 128, 512
C = 128
M_TOT = B * H * Wdim          # 65536
M_CORE = M_TOT // N_CORES     # 8192
KB = D // 128                 # 4 contraction blocks
M_CHUNK = 1024                # DMA chunk along M (2 MiB per chunk load)
M_SUB = 512                   # matmul moving size (one PSUM bank, fp32)

_NC = None


def _build():
    global _NC
    if _NC is not None:
        return _NC
    # Bacc (not raw Bass): its finalize() runs the pass pipeline that
    # splits multi-waits into EventSemaphores (TRN2 allows only one sync
    # wait per instruction) — Tile output does not compile without it.
    nc = bacc.Bacc(None)
    xt = nc.declare_dram_parameter(
        "xt", [KB, 128, M_CORE], mybir.dt.float32r, isOutput=False
    )
    wt = nc.declare_dram_parameter(
        "wt", [128, KB, C], mybir.dt.float32r, isOutput=False
    )
    bias = nc.declare_dram_parameter("bias", [C, 1], mybir.dt.float32, isOutput=False)
    out = nc.declare_dram_parameter("out", [C, M_CORE], mybir.dt.float32, isOutput=True)

    with tile.TileContext(nc) as tc:
        with (
            tc.tile_pool(name="const", bufs=1) as cpool,
            tc.tile_pool(name="x", bufs=6) as xpool,
            tc.tile_pool(name="o", bufs=4) as opool,
            tc.tile_pool(name="ps", bufs=4, space="PSUM") as pspool,
        ):
            # w/bias go over SWDGE (gpsimd) so the 8 HWDGE lanes map 1:1
            # onto the 8 big DMAs below.
            w_sb = cpool.tile([128, KB, C], mybir.dt.float32r)
            nc.gpsimd.dma_start(w_sb[:], wt[:])
            b_sb = cpool.tile([C, 1], mybir.dt.float32)
            nc.gpsimd.dma_start(b_sb[:], bias[:])

            xt_r = xt[:].rearrange("kb p m -> p kb m")
            for mc in range(M_CORE // M_CHUNK):
                x_sb = xpool.tile([128, KB, M_CHUNK], mybir.dt.float32r)
                nc.sync.dma_start(
                    x_sb[:], xt_r[:, :, mc * M_CHUNK : (mc + 1) * M_CHUNK]
                )
                o_sb = opool.tile([C, M_CHUNK], mybir.dt.float32)
                for ms in range(M_CHUNK // M_SUB):
                    ps = pspool.tile([C, M_SUB], mybir.dt.float32)
                    for kb in range(KB):
                        nc.tensor.matmul(
                            ps[:],
                            w_sb[:, kb, :],
                            x_sb[:, kb, bass.ts(ms, M_SUB)],
                            start=(kb == 0),
                            stop=(kb == KB - 1),
                        )
                    nc.vector.tensor_scalar_add(
                        o_sb[:, bass.ts(ms, M_SUB)], ps[:], b_sb[:]
                    )
                # Stores ride the ACT HWDGE ring so they never queue behind
                # the loads on the SP ring.
                nc.scalar.dma_start(
                    out[:, mc * M_CHUNK : (mc + 1) * M_CHUNK], o_sb[:]
                )
    nc.finalize()  # Bacc.finalize runs the wait-splitting compile pipeline
    _NC = nc
    return nc


LAST_RESULT = None


def kernel(x, W, b):
    global LAST_RESULT
    nc = _build()

    x = np.ascontiguousarray(np.asarray(x), dtype=np.float32)
    W = np.asarray(W, dtype=np.float32)
    b = np.asarray(b, dtype=np.float32)

    # Per-core K-major slabs: [8, D, M_CORE] -> [8, KB, 128, M_CORE]
    xt = np.ascontiguousarray(
        x.reshape(N_CORES, M_CORE, D).transpose(0, 2, 1)
    ).reshape(N_CORES, KB, 128, M_CORE)
    # Stationary weights, blocked: wt[kp, kb, c] = W[c, kb*128 + kp]
    wt = np.ascontiguousarray(W.T.reshape(KB, 128, C).transpose(1, 0, 2))
    b2 = np.ascontiguousarray(b.reshape(C, 1))

    in_maps = [{"xt": xt[i], "wt": wt, "bias": b2} for i in range(N_CORES)]
    res = run_bass_kernel_spmd(nc, in_maps, list(range(N_CORES)))
    LAST_RESULT = res

    out = np.empty((B, C, H, Wdim), dtype=np.float32)
    for i in range(N_CORES):
        slab = res.results[i]["out"]  # [C, M_CORE] over m = (h, w) for batch i//2
        bi, half = divmod(i, 2)
        out[bi, :, half * 64 : (half + 1) * 64, :] = slab.reshape(C, 64, Wdim)
    return out
